# revision 5
# baseline (speedup 1.0000x reference)
"""PatchMatch-style MatchingPropagator on 8 Trainium2 NeuronCores.

Full inputs in, full outputs out. Sharding: 8 independent units =
(direction in {forward, backward}) x (batch 0..3), one NeuronCore each.

Key layout decision: the host re-packs each unit's correlation volume into
"quad" records Q[n, y0, x0, 0:4] = corr[n, y0:y0+2, x0:x0+2] for anchors
(y0, x0) in [0,62]^2, so every bilinear sample needs exactly ONE contiguous
16-byte indirect-DMA fetch (the four corners) instead of two 8-byte pair
fetches.  Clamping floors to <=62 keeps x1=x0+1 / y1=y0+1 valid and is
numerically identical to the reference's corner clamping.

The device program mirrors the reference computation op-for-op in IEEE
fp32 so every propagate/random-search argmax decision matches the
reference bitwise.  All per-round element work runs as wide fused DVE ops
over strided views covering every candidate at once; the initial score
eval is folded into the first propagate's gather (candidates pre-rolled
on the host), so the chain is 7 gathers instead of 9.

Pixel layout on chip: pixel (i, j) -> partition 64*(j//32) + i, free j%32.
State per candidate block is [x(32) | y(32) | s(32)]; blocks are
[BEST | H | V] so a candidate acceptance is one predicated 96-col copy.
"""

import numpy as np

B, H, W = 4, 64, 64
R = 3.0
EPS = np.float32(0.01)
N_CORES = 8
PIX = H * W              # 4096 pixels per unit
AN = W - 1               # 63 anchors per axis in the quad layout
QROW = AN * 4            # 252 floats per anchor row
QMAP = AN * AN * 4       # 15876 floats per pixel quad map
M_RNE = float(1 << 23)

_CACHE = {}


# ----------------------------------------------------------------------------
# Device program (SPMD: identical on all 8 cores; data differs per core)
# ----------------------------------------------------------------------------

def _build_program():
    import concourse.bass as bass
    import concourse.mybir as mybir
    import concourse.tile as tile
    from concourse import bacc

    F32 = mybir.dt.float32
    I32 = mybir.dt.int32
    OP = mybir.AluOpType
    AF = mybir.ActivationFunctionType

    nc = bacc.Bacc(
        "TRN2",
        target_bir_lowering=False,
        debug=False,
        enable_asserts=False,
        num_devices=N_CORES,
    )

    corr = nc.dram_tensor("corr", [PIX * QMAP], F32, kind="ExternalInput")
    # state rows: 0 x, 1 y, 2 hx1, 3 hy1, 4 vx1, 5 vy1, 6 base, 7.. noise
    state_in = nc.dram_tensor("state", [13, 128, 32], F32,
                              kind="ExternalInput")
    out_xy = nc.dram_tensor("out_xy", [2, 128, 32], F32,
                            kind="ExternalOutput")

    corr_flat = corr.ap().rearrange("(n one) -> n one", one=1)

    def b3(ap):  # [128,32] -> broadcast [128,3,32]
        return ap.rearrange("p (one f) -> p one f", one=1).to_broadcast(
            [128, 3, 32])

    with tile.TileContext(nc) as tc:
        with tc.tile_pool(name="main", bufs=1) as pool:
            ST = pool.tile([128, 13 * 32], F32, name="ST")
            nc.sync.dma_start(
                ST[:].rearrange("p (n f) -> p n f", n=13),
                state_in.ap().rearrange("n p f -> p n f"),
            )
            BASE = ST[:, 192:224]

            def noise_view(k):
                o = 224 + 64 * k
                return ST[:, o:o + 64]  # [nx|ny]

            # CT blocks of 96: [BEST | H | V], each [x|y|s]
            CT = pool.tile([128, 288], F32, name="CT")
            G = pool.tile([128, 384], F32, name="G")
            WX = pool.tile([128, 192], F32, name="WX")   # [u w] interleaved
            TY = pool.tile([128, 192], F32, name="TY")   # [t wy] interleaved
            WF = pool.tile([128, 192], F32, name="WF")
            X0 = pool.tile([128, 192], F32, name="X0")
            IF = pool.tile([128, 96], F32, name="IF")
            I = pool.tile([128, 96], I32, name="I")
            A = pool.tile([128, 384], F32, name="A")
            Bt = pool.tile([128, 384], F32, name="Bt")
            S1 = pool.tile([128, 96], F32, name="S1")
            S2 = pool.tile([128, 96], F32, name="S2")
            UPD = pool.tile([128, 96], I32, name="UPD")

            v = nc.vector

            def blocks(nb, off=0):
                """[128, nb, 96] view of CT starting at block `off`."""
                return (CT[:]
                        .rearrange("p (b f) -> p b f", b=3)[:, off:off + nb])

            def emit_idx(nb, off=0):
                """floor+clamp+quad-index for blocks [off, off+nb) of CT.
                Writes X0/WF scratch and int32 indices I[:, 32*off:...]."""
                cv = blocks(nb, off)[:, :, 0:64]          # [128,nb,64] coords
                wf = WF[:].rearrange("p (b f) -> p b f", b=3)[:, :nb]
                x0 = X0[:].rearrange("p (b f) -> p b f", b=3)[:, :nb]
                v.tensor_scalar(wf, cv, M_RNE, M_RNE, OP.add, OP.subtract)
                v.tensor_tensor(x0, wf, cv, OP.is_gt)
                v.tensor_tensor(x0, wf, x0, OP.subtract)
                v.tensor_scalar(x0, x0, float(AN - 1), None, OP.min)
                xf = x0[:, :, 0:32]
                yf = x0[:, :, 32:64]
                if3 = IF[:, 32 * off:32 * (off + nb)].rearrange(
                    "p (b f) -> p b f", b=nb)
                i3 = I[:, 32 * off:32 * (off + nb)].rearrange(
                    "p (b f) -> p b f", b=nb)
                baseb = (BASE.rearrange("p (one f) -> p one f", one=1)
                         .to_broadcast([128, nb, 32]))
                v.scalar_tensor_tensor(if3, yf, float(QROW), baseb,
                                       OP.mult, OP.add)
                v.scalar_tensor_tensor(i3, xf, 4.0, if3, OP.mult, OP.add)

            def emit_gather(nb, off=0):
                nc.gpsimd.indirect_dma_start(
                    out=G[:, 0:128 * nb],
                    out_offset=None,
                    in_=corr_flat,
                    in_offset=bass.IndirectOffsetOnAxis(
                        ap=I[:, 32 * off:32 * (off + nb)], axis=0),
                )

            def emit_w(nb, off=0):
                """weights for blocks [off, off+nb): WX = [u|w], TY = [t|wy]
                interleaved per pixel.  Emitted after the gather dispatch so
                they hide under the DMA."""
                cv = blocks(nb, off)
                x0 = X0[:].rearrange("p (b f) -> p b f", b=3)[:, :nb]
                wxv = WX[:, 0:64 * nb].rearrange("p (b q s) -> p b q s", b=nb, s=2)
                tyv = TY[:, 0:64 * nb].rearrange("p (b q s) -> p b q s", b=nb, s=2)
                v.tensor_tensor(wxv[:, :, :, 1], cv[:, :, 0:32],
                                x0[:, :, 0:32], OP.subtract)
                v.tensor_tensor(tyv[:, :, :, 1], cv[:, :, 32:64],
                                x0[:, :, 32:64], OP.subtract)
                nc.scalar.activation(wxv[:, :, :, 0], wxv[:, :, :, 1],
                                     AF.Copy, bias=1.0, scale=-1.0)
                nc.scalar.activation(tyv[:, :, :, 0], tyv[:, :, :, 1],
                                     AF.Copy, bias=1.0, scale=-1.0)

            def emit_score(nb, off=0):
                """bilinear score for blocks [off, off+nb); scores land in
                CT block score columns.  Bit-exact against the reference:
                t_k = (corner*u_or_w)*t_or_wy, s = ((t1+t2)+t3)+t4."""
                g4 = G[:, 0:128 * nb].rearrange(
                    "p (b q dy dx) -> p b q dy dx", b=nb, dy=2, dx=2)
                wrep = (WX[:, 0:64 * nb]
                        .rearrange("p (b q one s) -> p b q one s", b=nb, one=1, s=2)
                        .to_broadcast([128, nb, 32, 2, 2]))
                trep = (TY[:, 0:64 * nb]
                        .rearrange("p (b q s one) -> p b q s one", b=nb, one=1, s=2)
                        .to_broadcast([128, nb, 32, 2, 2]))
                a4 = A[:, 0:128 * nb].rearrange(
                    "p (b q dy dx) -> p b q dy dx", b=nb, dy=2, dx=2)
                b4 = Bt[:, 0:128 * nb].rearrange(
                    "p (b q k) -> p b q k", b=nb, k=4)
                v.tensor_tensor(a4, g4, wrep, OP.mult)
                v.tensor_tensor(Bt[:, 0:128 * nb].rearrange(
                    "p (b q dy dx) -> p b q dy dx", b=nb, dy=2, dx=2),
                    a4, trep, OP.mult)
                s1 = S1[:, 0:32 * nb].rearrange("p (b f) -> p b f", b=nb)
                s2 = S2[:, 0:32 * nb].rearrange("p (b f) -> p b f", b=nb)
                sc = blocks(nb, off)[:, :, 64:96]
                v.tensor_tensor(s1, b4[:, :, :, 0], b4[:, :, :, 1], OP.add)
                v.tensor_tensor(s2, s1, b4[:, :, :, 2], OP.add)
                v.tensor_tensor(sc, s2, b4[:, :, :, 3], OP.add)

            def accept(src_off):
                """BEST = candidate block src_off where its score is higher."""
                so = 96 * src_off
                v.tensor_tensor(UPD[:], b3(CT[:, so + 64:so + 96]),
                                b3(CT[:, 64:96]), OP.is_gt)
                v.copy_predicated(CT[:, 0:96], UPD[:], CT[:, so:so + 96])

            # ---- round 1: initial eval + propagate(1,1), candidates
            # pre-rolled on the host
            v.tensor_copy(CT[:, 0:64], ST[:, 0:64])
            v.tensor_copy(CT[:, 96:160], ST[:, 64:128])
            v.tensor_copy(CT[:, 192:256], ST[:, 128:192])
            emit_idx(3)
            emit_gather(3)
            emit_w(3)
            emit_score(3)
            accept(1)
            accept(2)

            def propagate(dx, dy):
                # cand_v coords: row-roll of BEST [x|y] by dy via 2 fused-AP
                # DMAs (bulk + wrap) split across the two HWDGE issuers
                dv = CT[:, 192:256].rearrange("(b i) f -> b i f", b=2)
                sv = CT[:, 0:64].rearrange("(b i) f -> b i f", b=2)
                if dy == 1:
                    nc.sync.dma_start(dv[:, 1:64], sv[:, 0:63])
                    nc.scalar.dma_start(dv[:, 0:1], sv[:, 63:64])
                else:
                    nc.sync.dma_start(dv[:, 0:63], sv[:, 1:64])
                    nc.scalar.dma_start(dv[:, 63:64], sv[:, 0:1])

                # cand_h coords: col-roll by dx (DVE copies, overlap the DMA)
                dh = CT[:, 96:160].rearrange("p (c f) -> p c f", c=2)
                sh = CT[:, 0:64].rearrange("p (c f) -> p c f", c=2)
                if dx == 1:
                    v.tensor_copy(dh[:, :, 1:32], sh[:, :, 0:31])
                    v.tensor_copy(dh[64:128, :, 0:1], sh[0:64, :, 31:32])
                    v.tensor_copy(dh[0:64, :, 0:1], sh[64:128, :, 31:32])
                else:
                    v.tensor_copy(dh[:, :, 0:31], sh[:, :, 1:32])
                    v.tensor_copy(dh[0:64, :, 31:32], sh[64:128, :, 0:1])
                    v.tensor_copy(dh[64:128, :, 31:32], sh[0:64, :, 0:1])

                # shift + clamp (only the moving coordinate needs clamping)
                if dx == 1:
                    v.tensor_scalar(CT[:, 96:128], CT[:, 96:128], 1.0,
                                    float(W - 1), OP.add, OP.min)
                else:
                    v.tensor_scalar(CT[:, 96:128], CT[:, 96:128], -1.0, 0.0,
                                    OP.add, OP.max)
                if dy == 1:
                    v.tensor_scalar(CT[:, 224:256], CT[:, 224:256], 1.0,
                                    float(H - 1), OP.add, OP.min)
                else:
                    v.tensor_scalar(CT[:, 224:256], CT[:, 224:256], -1.0,
                                    0.0, OP.add, OP.max)

                emit_idx(2, 1)
                emit_gather(2, 1)
                emit_w(2, 1)
                emit_score(2, 1)
                accept(1)
                accept(2)

            def random_search(k):
                v.tensor_tensor(CT[:, 96:160], CT[:, 0:64], noise_view(k),
                                OP.add)
                v.tensor_scalar(CT[:, 96:160], CT[:, 96:160], 0.0,
                                float(W - 1), OP.max, OP.min)
                emit_idx(1, 1)
                emit_gather(1, 1)
                emit_w(1, 1)
                emit_score(1, 1)
                accept(1)

            random_search(0)
            propagate(-1, -1)
            random_search(1)
            propagate(-1, 1)
            random_search(2)
            propagate(1, -1)

            nc.sync.dma_start(
                out_xy.ap().rearrange("n p f -> p n f"),
                CT[:, 0:64].rearrange("p (n f) -> p n f", n=2),
            )

    nc.compile()
    return nc


def _get_program():
    if "nc" not in _CACHE:
        _CACHE["nc"] = _build_program()
    return _CACHE["nc"]


# ----------------------------------------------------------------------------
# Host-side helpers
# ----------------------------------------------------------------------------

def _to_layout(v):
    """[64(i), 64(j)] -> [128, 32]; partition = 64*(j//32)+i, free = j%32."""
    return np.ascontiguousarray(
        v.reshape(64, 2, 32).transpose(1, 0, 2).reshape(128, 32))


def _from_layout(a):
    """[128, 32] -> [64(i), 64(j)]."""
    return a.reshape(2, 64, 32).transpose(1, 0, 2).reshape(64, 64)


def _noise_arrays():
    """Mirror the reference's jax.random usage exactly, in-process, so the
    values match the grader's reference no matter which jax backend/PRNG
    the process defaults to."""
    import jax
    import jax.numpy as jnp

    key = jax.random.key(42)
    kf, kb = jax.random.split(key)
    out = []
    for kdir in (kf, kb):
        ks = jax.random.split(kdir, 3)
        out.append([np.asarray(R * jax.random.normal(k, (B, H, W, 2),
                                                     jnp.float32))
                    for k in ks])
    return out  # [dir][step] -> [B,H,W,2] float32


def _quad_pack(corr_u):
    """[4096, 64, 64] -> flat quad records [4096*63*63*4] f32."""
    sw = np.lib.stride_tricks.sliding_window_view(corr_u, (2, 2),
                                                  axis=(1, 2))
    # sw: [4096, 63, 63, 2, 2]
    return np.ascontiguousarray(sw).reshape(-1)


def _make_state(x_plane, y_plane, noise_steps, b):
    """Build the [13,128,32] per-core state tensor."""
    x = x_plane.astype(np.float32)
    y = y_plane.astype(np.float32)
    one = np.float32(1.0)
    # first propagate is (dx, dy) = (1, 1); host pre-rolls the candidates
    hx = np.clip(np.roll(x, 1, axis=1) + one, np.float32(0.0),
                 np.float32(W - 1))
    hy = np.roll(y, 1, axis=1)
    vx = np.roll(x, 1, axis=0)
    vy = np.clip(np.roll(y, 1, axis=0) + one, np.float32(0.0),
                 np.float32(H - 1))
    base = ((np.arange(64, dtype=np.int64)[:, None] * 64
             + np.arange(64, dtype=np.int64)[None, :]) * QMAP)
    rows = [
        _to_layout(x), _to_layout(y),
        _to_layout(hx), _to_layout(hy),
        _to_layout(vx), _to_layout(vy),
        _to_layout(base.astype(np.float32)),
    ]
    for step in range(3):
        nz = noise_steps[step][b]  # [H,W,2]
        rows.append(_to_layout(np.ascontiguousarray(nz[:, :, 0])))
        rows.append(_to_layout(np.ascontiguousarray(nz[:, :, 1])))
    return np.stack(rows).astype(np.float32)


def _bilinear_map_np(img, coords):
    """numpy mirror of reference._bilinear_map (fp32, same op order).
    img [B,H,W,C], coords [B,H,W,2] -> [B,H,W,C]"""
    Bn, Hn, Wn, C = img.shape
    out = np.empty_like(img)
    one = np.float32(1.0)
    for b in range(Bn):
        x = coords[b, :, :, 0].reshape(-1)
        y = coords[b, :, :, 1].reshape(-1)
        x0 = np.floor(x)
        y0 = np.floor(y)
        wx = (x - x0)[:, None]
        wy = (y - y0)[:, None]
        x0i = np.clip(x0.astype(np.int32), 0, Wn - 1)
        x1i = np.clip(x0i + 1, 0, Wn - 1)
        y0i = np.clip(y0.astype(np.int32), 0, Hn - 1)
        y1i = np.clip(y0i + 1, 0, Hn - 1)
        im = img[b]
        v00 = im[y0i, x0i]
        v01 = im[y0i, x1i]
        v10 = im[y1i, x0i]
        v11 = im[y1i, x1i]
        o = (v00 * (one - wx) * (one - wy) + v01 * wx * (one - wy)
             + v10 * (one - wx) * wy + v11 * wx * wy)
        out[b] = o.reshape(Hn, Wn, C)
    return out


def _run_device(in_maps, trace=False):
    from concourse import bass_utils

    nc = _get_program()
    res = bass_utils.run_bass_kernel_spmd(
        nc, in_maps, core_ids=list(range(N_CORES)), trace=trace)
    return res


def kernel(matching_f, matching_b, corr_map, _trace=False, _results_hook=None):
    matching_f = np.asarray(matching_f)
    matching_b = np.asarray(matching_b)
    corr_map = np.asarray(corr_map)

    noise = _noise_arrays()  # [dir][step][B,H,W,2]

    in_maps = []
    for b in range(B):  # forward units, cores 0..3
        corr_u = np.ascontiguousarray(corr_map[b]).reshape(PIX, H, W)
        in_maps.append({
            "corr": _quad_pack(corr_u),
            "state": _make_state(matching_f[b, 0], matching_f[b, 1],
                                 noise[0], b),
        })
    for b in range(B):  # backward units, cores 4..7
        corr_t = np.ascontiguousarray(
            corr_map[b].transpose(2, 3, 0, 1)).reshape(PIX, H, W)
        in_maps.append({
            "corr": _quad_pack(corr_t),
            "state": _make_state(matching_b[b, 0], matching_b[b, 1],
                                 noise[1], b),
        })

    res = _run_device(in_maps, trace=_trace)
    if _results_hook is not None:
        _results_hook(res)

    res_f = np.empty((B, H, W, 2), np.float32)
    res_b = np.empty((B, H, W, 2), np.float32)
    for b in range(B):
        of = res.results[b]["out_xy"]
        ob = res.results[4 + b]["out_xy"]
        res_f[b, :, :, 0] = _from_layout(of[0])
        res_f[b, :, :, 1] = _from_layout(of[1])
        res_b[b, :, :, 0] = _from_layout(ob[0])
        res_b[b, :, :, 1] = _from_layout(ob[1])

    # forward-backward consistency (host; mirrors reference in fp32)
    counter = _bilinear_map_np(res_b, res_f)
    diff = np.max(np.abs(res_f - counter), axis=-1)
    invalid = (diff > EPS)[..., None]
    mf_t = matching_f.transpose(0, 2, 3, 1)  # [B,H,W,2]
    out = np.where(invalid, mf_t, res_f)
    return np.ascontiguousarray(out.transpose(0, 3, 1, 2)).astype(np.float32)


# revision 7
# speedup vs baseline: 1.0789x; 1.0789x over previous
"""PatchMatch-style MatchingPropagator on 8 Trainium2 NeuronCores.

Full inputs in, full outputs out. Sharding: 8 independent units =
(direction in {forward, backward}) x (batch 0..3), one NeuronCore each.

Key layout decisions:
- The host re-packs each unit's correlation volume into "quad" records
  Q[n, y0, x0, 0:4] = corr[n, y0:y0+2, x0:x0+2] for anchors in [0,62]^2,
  so every bilinear sample is ONE contiguous 16-byte indirect-DMA fetch.
  Clamping floors to <=62 is numerically identical to the reference's
  corner clamping.
- Every DVE op on the critical path reads/writes contiguous (or at most
  3-dim strided) access patterns; measured on TRN2, deep strided/broadcast
  views cost 2-3x a contiguous op of the same size.
- Candidate coords live in CC = [x-cols | y-cols] so floor/clamp/index
  ops are single wide contiguous ops; the [x|y|s] accept blocks in CT are
  filled by copies hidden under the gather's DMA flight time.
- The score uses prebuilt interleaved weight tiles UW = [u w u w] and
  TW = [t t wy wy] per pixel (built off the critical path), so the score
  is 2 contiguous multiplies + 3 stride-4 adds, bit-exact against the
  reference's product/sum order: s = ((t1+t2)+t3)+t4.
- The initial score eval is folded into the first propagate's gather
  (candidates pre-rolled on the host): 7 gathers total.

Pixel layout on chip: pixel (i, j) -> partition 64*(j//32) + i, free j%32.
"""

import numpy as np

B, H, W = 4, 64, 64
R = 3.0
EPS = np.float32(0.01)
N_CORES = 8
PIX = H * W              # 4096 pixels per unit
AN = W - 1               # 63 anchors per axis in the quad layout
QROW = AN * 4            # 252 floats per anchor row
QMAP = AN * AN * 4       # 15876 floats per pixel quad map
M_RNE = float(1 << 23)

_CACHE = {}


# ----------------------------------------------------------------------------
# Device program (SPMD: identical on all 8 cores; data differs per core)
# ----------------------------------------------------------------------------

def _build_program():
    import concourse.bass as bass
    import concourse.mybir as mybir
    import concourse.tile as tile
    from concourse import bacc

    F32 = mybir.dt.float32
    I32 = mybir.dt.int32
    OP = mybir.AluOpType
    AF = mybir.ActivationFunctionType

    nc = bacc.Bacc(
        "TRN2",
        target_bir_lowering=False,
        debug=False,
        enable_asserts=False,
        num_devices=N_CORES,
    )

    corr = nc.dram_tensor("corr", [PIX * QMAP], F32, kind="ExternalInput")
    # state rows: [x, y, hx1, hy1, vx1, vy1, base, nx1, ny1, nx2, ny2,
    #              nx3, ny3]
    state_in = nc.dram_tensor("state", [13, 128, 32], F32,
                              kind="ExternalInput")
    out_xy = nc.dram_tensor("out_xy", [2, 128, 32], F32,
                            kind="ExternalOutput")

    corr_flat = corr.ap().rearrange("(n one) -> n one", one=1)

    def b3(ap):  # [128,32] -> broadcast [128,3,32]
        return ap.rearrange("p (one f) -> p one f", one=1).to_broadcast(
            [128, 3, 32])

    with tile.TileContext(nc) as tc:
        with tc.tile_pool(name="main", bufs=1) as pool:
            ST = pool.tile([128, 13 * 32], F32, name="ST")
            nc.sync.dma_start(
                ST[:].rearrange("p (n f) -> p n f", n=13),
                state_in.ap().rearrange("n p f -> p n f"),
            )
            BASE = ST[:, 192:224]

            def noise_view(k):
                o = 224 + 64 * k
                return ST[:, o:o + 64]  # [nx|ny]

            # CT accept blocks of 96: [BEST | H | V], each [x|y|s]
            CT = pool.tile([128, 288], F32, name="CT")
            # CC: contiguous candidate coords [x-cols | y-cols]
            CC = pool.tile([128, 128], F32, name="CC")
            G = pool.tile([128, 384], F32, name="G")
            UW = pool.tile([128, 384], F32, name="UW")   # [u w u w] per px
            TW = pool.tile([128, 384], F32, name="TW")   # [t t wy wy] per px
            WT = pool.tile([128, 192], F32, name="WT")   # [w-cols | wy-cols]
            WF = pool.tile([128, 192], F32, name="WF")
            X0 = pool.tile([128, 192], F32, name="X0")
            IF = pool.tile([128, 96], F32, name="IF")
            I = pool.tile([128, 96], I32, name="I")
            B1 = pool.tile([128, 384], F32, name="B1")
            B2 = pool.tile([128, 384], F32, name="B2")
            S1 = pool.tile([128, 96], F32, name="S1")
            S2 = pool.tile([128, 96], F32, name="S2")
            UPD = pool.tile([128, 96], I32, name="UPD")

            v = nc.vector

            def do_floor(cv, n):
                """floor+clamp of n contiguous coord cols into X0[:, 0:n]."""
                wf = WF[:, 0:n]
                x0 = X0[:, 0:n]
                v.tensor_scalar(wf, cv, M_RNE, M_RNE, OP.add, OP.subtract)
                v.tensor_tensor(x0, wf, cv, OP.is_gt)
                v.tensor_tensor(x0, wf, x0, OP.subtract)
                v.tensor_scalar(x0, x0, float(AN - 1), None, OP.min)

            def do_idx(ne):
                """quad-record indices from X0 = [xf yf] pairs per
                candidate; writes I[:, 0:32*ne] in (cand, q) order."""
                m = 32 * ne
                x2 = X0[:, 0:2 * m].rearrange("p (c s q) -> p c s q",
                                              c=ne, s=2)
                xf = x2[:, :, 0]
                yf = x2[:, :, 1]
                if3 = IF[:, 0:m].rearrange("p (e q) -> p e q", e=ne)
                i3 = I[:, 0:m].rearrange("p (e q) -> p e q", e=ne)
                baseb = (BASE.rearrange("p (one f) -> p one f", one=1)
                         .to_broadcast([128, ne, 32]))
                v.scalar_tensor_tensor(if3, yf, float(QROW), baseb,
                                       OP.mult, OP.add)
                v.scalar_tensor_tensor(i3, xf, 4.0, if3, OP.mult, OP.add)

            def do_gather(ne):
                nc.gpsimd.indirect_dma_start(
                    out=G[:, 0:128 * ne],
                    out_offset=None,
                    in_=corr_flat,
                    in_offset=bass.IndirectOffsetOnAxis(
                        ap=I[:, 0:32 * ne], axis=0),
                )

            def do_weights(cv, ne):
                """hidden under gather flight: build UW = [u w u w] and
                TW = [t t wy wy] per pixel from coords and floors.
                cv: [128, 64*ne] coords as [x y] pairs per candidate."""
                m = 32 * ne
                c2 = cv.rearrange("p (c s q) -> p c s q", c=ne, s=2)
                x2 = X0[:, 0:2 * m].rearrange("p (c s q) -> p c s q",
                                              c=ne, s=2)
                w = WT[:, 0:m].rearrange("p (e q) -> p e q", e=ne)
                wy = WT[:, m:2 * m].rearrange("p (e q) -> p e q", e=ne)
                v.tensor_tensor(w, c2[:, :, 0], x2[:, :, 0], OP.subtract)
                v.tensor_tensor(wy, c2[:, :, 1], x2[:, :, 1], OP.subtract)
                w = WT[:, 0:m]
                wy = WT[:, m:2 * m]
                uwv = UW[:, 0:128 * ne].rearrange(
                    "p (e d s) -> p e d s", e=m, d=2, s=2)
                twv = TW[:, 0:128 * ne].rearrange(
                    "p (e s d) -> p e s d", e=m, s=2, d=2)
                wb = (w.rearrange("p (e one) -> p e one", one=1)
                      .to_broadcast([128, m, 2]))
                wyb = (wy.rearrange("p (e one) -> p e one", one=1)
                       .to_broadcast([128, m, 2]))
                v.tensor_copy(uwv[:, :, :, 1], wb)
                v.tensor_copy(twv[:, :, 1, :], wyb)
                nc.scalar.activation(uwv[:, :, :, 0], uwv[:, :, :, 1],
                                     AF.Copy, bias=1.0, scale=-1.0)
                nc.scalar.activation(twv[:, :, 0, :], twv[:, :, 1, :],
                                     AF.Copy, bias=1.0, scale=-1.0)

            def do_score(ne, sc_dst):
                """bilinear score; bit-exact term/sum order of the
                reference: t_k = (corner*u_or_w)*t_or_wy,
                s = ((t1+t2)+t3)+t4."""
                m = 32 * ne
                v.tensor_tensor(B1[:, 0:128 * ne], G[:, 0:128 * ne],
                                UW[:, 0:128 * ne], OP.mult)
                v.tensor_tensor(B2[:, 0:128 * ne], B1[:, 0:128 * ne],
                                TW[:, 0:128 * ne], OP.mult)
                b4 = B2[:, 0:128 * ne].rearrange("p (e k) -> p e k", k=4)
                s1 = S1[:, 0:m].rearrange("p (e one) -> p e one", one=1)
                s2 = S2[:, 0:m].rearrange("p (e one) -> p e one", one=1)
                v.tensor_tensor(s1, b4[:, :, 0:1], b4[:, :, 1:2], OP.add)
                v.tensor_tensor(s2, s1, b4[:, :, 2:3], OP.add)
                v.tensor_tensor(sc_dst, s2, b4[:, :, 3:4], OP.add)

            def accept(blk):
                """BEST = candidate block blk where its score is higher."""
                so = 96 * blk
                v.tensor_tensor(UPD[:], b3(CT[:, so + 64:so + 96]),
                                b3(CT[:, 64:96]), OP.is_gt)
                v.copy_predicated(CT[:, 0:96], UPD[:], CT[:, so:so + 96])

            def sc3(nb):
                """CT score-column view [128, nb, 32] for blocks 1..nb."""
                return (CT[:].rearrange("p (b f) -> p b f", b=3)
                        [:, 1:1 + nb, 64:96])

            # ---- round 1: initial eval + propagate(1,1); candidate coords
            # pre-rolled on the host, laid out as [x hx vx | y hy vy]
            do_floor(ST[:, 0:192], 192)
            do_idx(3)
            do_gather(3)
            do_weights(ST[:, 0:192], 3)
            v.tensor_copy(CT[:, 0:64], ST[:, 0:64])
            v.tensor_copy(CT[:, 96:160], ST[:, 64:128])
            v.tensor_copy(CT[:, 192:256], ST[:, 128:192])
            # scores: e-order is (b, h, v) -> CT blocks 0..2 score cols
            do_score(3, (CT[:].rearrange("p (b f) -> p b f", b=3)
                         [:, :, 64:96]))
            accept(1)
            accept(2)

            def propagate(dx, dy):
                # cand_v coords: row-roll of BEST [x|y] by dy via 2 fused-AP
                # DMAs (bulk + wrap) on the two HWDGE issuers
                dvv = CC[:, 64:128].rearrange("(b i) f -> b i f", b=2)
                svv = CT[:, 0:64].rearrange("(b i) f -> b i f", b=2)
                if dy == 1:
                    nc.sync.dma_start(dvv[:, 1:64], svv[:, 0:63])
                    nc.scalar.dma_start(dvv[:, 0:1], svv[:, 63:64])
                else:
                    nc.sync.dma_start(dvv[:, 0:63], svv[:, 1:64])
                    nc.scalar.dma_start(dvv[:, 63:64], svv[:, 0:1])

                # cand_h coords: col-roll by dx (DVE copies, overlap the DMA)
                dh = CC[:, 0:64].rearrange("p (c f) -> p c f", c=2)
                sh = CT[:, 0:64].rearrange("p (c f) -> p c f", c=2)
                if dx == 1:
                    v.tensor_copy(dh[:, :, 1:32], sh[:, :, 0:31])
                    v.tensor_copy(dh[64:128, :, 0:1], sh[0:64, :, 31:32])
                    v.tensor_copy(dh[0:64, :, 0:1], sh[64:128, :, 31:32])
                else:
                    v.tensor_copy(dh[:, :, 0:31], sh[:, :, 1:32])
                    v.tensor_copy(dh[0:64, :, 31:32], sh[64:128, :, 0:1])
                    v.tensor_copy(dh[64:128, :, 31:32], sh[0:64, :, 0:1])

                # shift + clamp (only the moving coordinate needs clamping)
                if dx == 1:
                    v.tensor_scalar(CC[:, 0:32], CC[:, 0:32], 1.0,
                                    float(W - 1), OP.add, OP.min)
                else:
                    v.tensor_scalar(CC[:, 0:32], CC[:, 0:32], -1.0, 0.0,
                                    OP.add, OP.max)
                if dy == 1:
                    v.tensor_scalar(CC[:, 96:128], CC[:, 96:128], 1.0,
                                    float(H - 1), OP.add, OP.min)
                else:
                    v.tensor_scalar(CC[:, 96:128], CC[:, 96:128], -1.0,
                                    0.0, OP.add, OP.max)

                do_floor(CC[:, 0:128], 128)
                do_idx(2)
                do_gather(2)
                do_weights(CC[:, 0:128], 2)
                # fill CT candidate coord cols (hidden under gather flight)
                v.tensor_copy(CT[:, 96:160], CC[:, 0:64])
                v.tensor_copy(CT[:, 192:256], CC[:, 64:128])
                do_score(2, sc3(2))
                accept(1)
                accept(2)

            def random_search(k):
                v.tensor_tensor(CC[:, 0:64], CT[:, 0:64], noise_view(k),
                                OP.add)
                v.tensor_scalar(CC[:, 0:64], CC[:, 0:64], 0.0,
                                float(W - 1), OP.max, OP.min)
                do_floor(CC[:, 0:64], 64)
                do_idx(1)
                do_gather(1)
                do_weights(CC[:, 0:64], 1)
                v.tensor_copy(CT[:, 96:160], CC[:, 0:64])
                do_score(1, sc3(1))
                accept(1)

            random_search(0)
            propagate(-1, -1)
            random_search(1)
            propagate(-1, 1)
            random_search(2)
            propagate(1, -1)

            nc.sync.dma_start(
                out_xy.ap().rearrange("n p f -> p n f"),
                CT[:, 0:64].rearrange("p (n f) -> p n f", n=2),
            )

    nc.compile()
    return nc


def _get_program():
    if "nc" not in _CACHE:
        _CACHE["nc"] = _build_program()
    return _CACHE["nc"]


# ----------------------------------------------------------------------------
# Host-side helpers
# ----------------------------------------------------------------------------

def _to_layout(v):
    """[64(i), 64(j)] -> [128, 32]; partition = 64*(j//32)+i, free = j%32."""
    return np.ascontiguousarray(
        v.reshape(64, 2, 32).transpose(1, 0, 2).reshape(128, 32))


def _from_layout(a):
    """[128, 32] -> [64(i), 64(j)]."""
    return a.reshape(2, 64, 32).transpose(1, 0, 2).reshape(64, 64)


def _noise_arrays():
    """Mirror the reference's jax.random usage exactly, in-process, so the
    values match the grader's reference no matter which jax backend/PRNG
    the process defaults to."""
    import jax
    import jax.numpy as jnp

    key = jax.random.key(42)
    kf, kb = jax.random.split(key)
    out = []
    for kdir in (kf, kb):
        ks = jax.random.split(kdir, 3)
        out.append([np.asarray(R * jax.random.normal(k, (B, H, W, 2),
                                                     jnp.float32))
                    for k in ks])
    return out  # [dir][step] -> [B,H,W,2] float32


def _quad_pack(corr_u):
    """[4096, 64, 64] -> flat quad records [4096*63*63*4] f32."""
    sw = np.lib.stride_tricks.sliding_window_view(corr_u, (2, 2),
                                                  axis=(1, 2))
    # sw: [4096, 63, 63, 2, 2]
    return np.ascontiguousarray(sw).reshape(-1)


def _make_state(x_plane, y_plane, noise_steps, b):
    """Build the [13,128,32] per-core state tensor."""
    x = x_plane.astype(np.float32)
    y = y_plane.astype(np.float32)
    one = np.float32(1.0)
    # first propagate is (dx, dy) = (1, 1); host pre-rolls the candidates
    hx = np.clip(np.roll(x, 1, axis=1) + one, np.float32(0.0),
                 np.float32(W - 1))
    hy = np.roll(y, 1, axis=1)
    vx = np.roll(x, 1, axis=0)
    vy = np.clip(np.roll(y, 1, axis=0) + one, np.float32(0.0),
                 np.float32(H - 1))
    base = ((np.arange(64, dtype=np.int64)[:, None] * 64
             + np.arange(64, dtype=np.int64)[None, :]) * QMAP)
    rows = [
        _to_layout(x), _to_layout(y),
        _to_layout(hx), _to_layout(hy),
        _to_layout(vx), _to_layout(vy),
        _to_layout(base.astype(np.float32)),
    ]
    for step in range(3):
        nz = noise_steps[step][b]  # [H,W,2]
        rows.append(_to_layout(np.ascontiguousarray(nz[:, :, 0])))
        rows.append(_to_layout(np.ascontiguousarray(nz[:, :, 1])))
    return np.stack(rows).astype(np.float32)


def _bilinear_map_np(img, coords):
    """numpy mirror of reference._bilinear_map (fp32, same op order).
    img [B,H,W,C], coords [B,H,W,2] -> [B,H,W,C]"""
    Bn, Hn, Wn, C = img.shape
    out = np.empty_like(img)
    one = np.float32(1.0)
    for b in range(Bn):
        x = coords[b, :, :, 0].reshape(-1)
        y = coords[b, :, :, 1].reshape(-1)
        x0 = np.floor(x)
        y0 = np.floor(y)
        wx = (x - x0)[:, None]
        wy = (y - y0)[:, None]
        x0i = np.clip(x0.astype(np.int32), 0, Wn - 1)
        x1i = np.clip(x0i + 1, 0, Wn - 1)
        y0i = np.clip(y0.astype(np.int32), 0, Hn - 1)
        y1i = np.clip(y0i + 1, 0, Hn - 1)
        im = img[b]
        v00 = im[y0i, x0i]
        v01 = im[y0i, x1i]
        v10 = im[y1i, x0i]
        v11 = im[y1i, x1i]
        o = (v00 * (one - wx) * (one - wy) + v01 * wx * (one - wy)
             + v10 * (one - wx) * wy + v11 * wx * wy)
        out[b] = o.reshape(Hn, Wn, C)
    return out


def _run_device(in_maps, trace=False):
    from concourse import bass_utils

    nc = _get_program()
    res = bass_utils.run_bass_kernel_spmd(
        nc, in_maps, core_ids=list(range(N_CORES)), trace=trace)
    return res


def kernel(matching_f, matching_b, corr_map, _trace=False, _results_hook=None):
    matching_f = np.asarray(matching_f)
    matching_b = np.asarray(matching_b)
    corr_map = np.asarray(corr_map)

    noise = _noise_arrays()  # [dir][step][B,H,W,2]

    in_maps = []
    for b in range(B):  # forward units, cores 0..3
        corr_u = np.ascontiguousarray(corr_map[b]).reshape(PIX, H, W)
        in_maps.append({
            "corr": _quad_pack(corr_u),
            "state": _make_state(matching_f[b, 0], matching_f[b, 1],
                                 noise[0], b),
        })
    for b in range(B):  # backward units, cores 4..7
        corr_t = np.ascontiguousarray(
            corr_map[b].transpose(2, 3, 0, 1)).reshape(PIX, H, W)
        in_maps.append({
            "corr": _quad_pack(corr_t),
            "state": _make_state(matching_b[b, 0], matching_b[b, 1],
                                 noise[1], b),
        })

    res = _run_device(in_maps, trace=_trace)
    if _results_hook is not None:
        _results_hook(res)

    res_f = np.empty((B, H, W, 2), np.float32)
    res_b = np.empty((B, H, W, 2), np.float32)
    for b in range(B):
        of = res.results[b]["out_xy"]
        ob = res.results[4 + b]["out_xy"]
        res_f[b, :, :, 0] = _from_layout(of[0])
        res_f[b, :, :, 1] = _from_layout(of[1])
        res_b[b, :, :, 0] = _from_layout(ob[0])
        res_b[b, :, :, 1] = _from_layout(ob[1])

    # forward-backward consistency (host; mirrors reference in fp32)
    counter = _bilinear_map_np(res_b, res_f)
    diff = np.max(np.abs(res_f - counter), axis=-1)
    invalid = (diff > EPS)[..., None]
    mf_t = matching_f.transpose(0, 2, 3, 1)  # [B,H,W,2]
    out = np.where(invalid, mf_t, res_f)
    return np.ascontiguousarray(out.transpose(0, 3, 1, 2)).astype(np.float32)
